# revision 18
# baseline (speedup 1.0000x reference)
"""Deformable conv (3x3, with offset-predicting conv) for Trainium2, 8 cores.

Sharding: pure data parallel. Core k handles sample b = k//2, output row block
(k%2)*48 .. +48 (48 rows x 96 cols = 4608 pixels). Full sample's x is available
to every core as a DRAM row-pair token table, so gathers are purely local.

Per-core pipeline (all on one NeuronCore, scheduled by Tile):
  A. offset conv (3x3, C=256 -> 18) as 18 PE matmuls per 4-row chunk
  B. PE-transpose offsets into pixel-partition layout [128px, tile, 18]
  C-w. DVE weight math in pixel layout: 4 bilinear corner weights with
     zero-pad semantics exactly like the reference -> wt[:, t, tap, 4]
  D. fold offsets (f32, 72B beats) into the SWDGE "wrapped 16-partition"
     layout doffW0[r, (t, m, 18)] via 8 DMAs; compute gather base indices
     directly in wrapped layout on the DVE; int16-copy into the
     (s, tap, t', m) stream order; replicate x8.
  E. per (stage of 512 px, tap): ONE dma_gather of 2KB 4-corner descriptors
     from the row-pair table xt2 -> [128px, tile, 4*C]; DVE 4-term FMA
     combine with per-partition (per-pixel) scalar weights; PE transpose to
     channel layout; PE matmul accumulating over (c,tap) into PSUM [o,px];
     bias + store.

The 4-corner trick: xt2[i] = concat(xflat[i], xflat[i+96]) over a zero-padded
token space (1 pad row above, 3 below), so one contiguous 1024-element read at
row base2 = (y0+1)*96 + xb yields all four bilinear corners
(y0,xb),(y1,xb),(y0,xb+1),(y1,xb+1).
"""

import dataclasses

import numpy as np

import concourse.bacc as bacc
import concourse.bass as bass
import concourse.mybir as mybir
import concourse.tile as tile
from concourse import bass_utils, masks
from concourse.mybir import ActivationFunctionType as Act
from concourse.mybir import AluOpType as Op

P = 128
B, C, H, W, O = 4, 256, 96, 96, 256
K = 3
K2 = 9
NCORES = 8
ROWS = 48                      # output rows per core
NPIX = ROWS * W                # 4608
NTILE = NPIX // P              # 36 pixel tiles of 128
NSTAGE = 9                     # stages of 512 px
TPS = 4                        # pixel tiles per stage
SPX = TPS * P                  # 512
PADH, PADW = ROWS + 2, W + 2   # 50, 98
XT2R = 9412                    # row-pair table rows (max base2 9407, +1, pad)
XPAD = 9600                    # padded flat token space (96 above, 288 below)
CONV_ROWS_PER_CHUNK = 4        # offset-conv N chunk = 4 rows = 384 cols
NCHUNK = ROWS // CONV_ROWS_PER_CHUNK  # 12
NW = NTILE * 8 * K2            # 2592 wrapped-layout elems per idx partition
BF = mybir.dt.bfloat16
F32 = mybir.dt.float32
I16 = mybir.dt.int16

_BUILT = {}


def _emit(tc, nc, io):
    xt2, xc, wofl, boff, wdcl, bdc, pyb, pxb, pybw, pxbw, out = io

    with (
        tc.tile_pool(name="const", bufs=1) as cpool,
        tc.tile_pool(name="sbig", bufs=1) as spool,
    ):
        ident_bf = cpool.tile([P, P], BF, tag="idbf", name="idbf")
        ident_f = cpool.tile([P, P], F32, tag="idf", name="idf")
        masks.make_identity(nc, ident_bf[:])
        masks.make_identity(nc, ident_f[:])

        # ---- persistent SBUF buffers ----
        xc_sb = spool.tile([P, 2, PADH * PADW], BF, tag="xc", name="xc")      # 19.6KB/part
        wofl_sb = spool.tile([P, 2, K2, 18], BF, tag="wofl", name="wofl")
        wdcl_sb = spool.tile([P, K2, 2, 2, P], BF, tag="wdcl", name="wdcl")     # 9.2KB/part
        boff_sb = spool.tile([18, 1], F32, tag="boff", name="boff")
        bdc_sb = spool.tile([P, 2], F32, tag="bdc", name="bdc")
        pyb_sb = spool.tile([P, NTILE, K2], F32, tag="pyb", name="pyb")
        pxb_sb = spool.tile([P, NTILE, K2], F32, tag="pxb", name="pxb")
        pybw_sb = spool.tile([16, NTILE, 8, K2], F32, tag="pybw", name="pybw")
        pxbw_sb = spool.tile([16, NTILE, 8, K2], F32, tag="pxbw", name="pxbw")
        off_sb = spool.tile([18, NPIX], F32, tag="off", name="off")            # 18 parts
        doff = spool.tile([P, NTILE, 18], F32, tag="doff", name="doff")
        doffw = spool.tile([16, NTILE, 8, 18], F32, tag="doffw", name="doffw")  # 20.7KB
        wt = spool.tile([P, NTILE, K2, 4], F32, tag="wt", name="wt")          # corner wgts
        twrap = spool.tile([P, NSTAGE, K2, TPS, 8], I16, tag="twrap", name="twrap")  # 5.2KB

        nc.sync.dma_start(xc_sb[:], xc)
        nc.sync.dma_start(wofl_sb[:], wofl)
        nc.sync.dma_start(wdcl_sb[:], wdcl)
        nc.sync.dma_start(boff_sb[:], boff)
        nc.sync.dma_start(bdc_sb[:], bdc)
        nc.sync.dma_start(pyb_sb[:], pyb)
        nc.sync.dma_start(pxb_sb[:], pxb)
        nc.sync.dma_start(pybw_sb[:], pybw)
        nc.sync.dma_start(pxbw_sb[:], pxbw)

        # ---- A/B/C/D pipelined per group of 4 conv chunks (12 tiles) ----
        # A: offset conv chunk (4 rows); B: PE-transpose its 3 tiles; after
        # each group of 4 chunks: C-w corner weights for those 12 tiles, fold
        # offsets into wrapped layout, then per-stage index math + replicate,
        # so stage-0 gathers can start while later conv chunks still run.
        MAGIC = 8388608.0  # 2^23
        with (
            tc.tile_pool(name="psA", bufs=2, space="PSUM") as psa,
            tc.tile_pool(name="psB", bufs=4, space="PSUM") as psb,
            tc.tile_pool(name="scr", bufs=1) as scr,
            tc.tile_pool(name="scw", bufs=1) as scw,
        ):
            sh = [P, NTILE, K2]

            def tmp(tag):
                return scr.tile(sh, F32, tag=tag, name=tag)

            pyw = scw.tile([16, NTILE, 8, K2], F32, tag="pyw", name="pyw")
            pxw = scw.tile([16, NTILE, 8, K2], F32, tag="pxw", name="pxw")

            for ch_i in range(NCHUNK):
                ncols = CONV_ROWS_PER_CHUNK * W  # 384
                ps = psa.tile([18, ncols], F32, tag="psoff", name="psoff")
                n_mm = 2 * K2
                mm = 0
                xcf = xc_sb[:]
                for chalf in range(2):
                    for tap in range(K2):
                        ti, tj = tap // K, tap % K
                        rhs = dataclasses.replace(
                            xcf,
                            ap=[
                                [xcf.ap[0][0], P],
                                [PADW, CONV_ROWS_PER_CHUNK],
                                [1, W],
                            ],
                            offset=xcf.offset
                            + chalf * (PADH * PADW)
                            + ((ch_i * CONV_ROWS_PER_CHUNK + ti) * PADW + tj),
                        )
                        nc.tensor.matmul(
                            ps[:],
                            wofl_sb[:, chalf, tap],
                            rhs,
                            start=(mm == 0),
                            stop=(mm == n_mm - 1),
                        )
                        mm += 1
                nc.scalar.activation(
                    off_sb[:, ch_i * ncols : (ch_i + 1) * ncols],
                    ps[:],
                    Act.Identity,
                    bias=boff_sb[:],
                )
                # B: transpose this chunk's 3 tiles to pixel layout
                for t in range(3 * ch_i, 3 * ch_i + 3):
                    pt = psb.tile([P, 18], F32, tag="pofft", name="pofft")
                    nc.tensor.transpose(
                        pt[:], off_sb[:, t * P : (t + 1) * P], ident_f[:18, :18]
                    )
                    nc.scalar.copy(doff[:, t, :], pt[:])

                # ---- per-stage fold + index math as soon as the stage's 4
                # tiles exist (stage s = tiles 4s..4s+4; chunk c = tiles
                # 3c..3c+3). Emitted ahead of C-w so stage-0 gathers can
                # start ~30us earlier.
                for s in range(NSTAGE):
                    if (4 * s + 3) // 3 != ch_i:
                        continue
                    # fold: doffw[r, t, m, ch] = doff[16m + r, t, ch]
                    for m in range(8):
                        nc.sync.dma_start(
                            doffw[0:16, 4 * s : 4 * s + 4, m, :],
                            doff[16 * m : 16 * m + 16, 4 * s : 4 * s + 4, :],
                        )
                    # base2 = (clip(y0,-1,96)+1)*96 + clip(x0,0,95), +16
                    # shift and floor bias baked into pybw/pxbw.
                    sl = slice(TPS * s, TPS * (s + 1))
                    pys = pyw[:, sl]
                    pxs = pxw[:, sl]
                    nc.vector.tensor_tensor(
                        pys, pybw_sb[:, sl], doffw[:, sl, :, 0:18:2], Op.add
                    )
                    nc.vector.tensor_tensor(
                        pxs, pxbw_sb[:, sl], doffw[:, sl, :, 1:18:2], Op.add
                    )
                    nc.vector.tensor_scalar(pys, pys, MAGIC, -MAGIC, Op.add, Op.add)
                    nc.vector.tensor_scalar(pxs, pxs, MAGIC, -MAGIC, Op.add, Op.add)
                    nc.vector.tensor_scalar(pys, pys, 15.0, 112.0, Op.max, Op.min)
                    nc.vector.tensor_scalar(pxs, pxs, 16.0, 111.0, Op.max, Op.min)
                    nc.vector.tensor_scalar(pys, pys, 96.0, -1456.0, Op.mult, Op.add)
                    nc.vector.tensor_tensor(pys, pys, pxs, Op.add)
                    nc.vector.tensor_copy(
                        twrap[0:16, s].transpose((0, 2, 3, 1)), pys
                    )
                    for g in range(1, 8):
                        nc.sync.dma_start(
                            twrap[16 * g : 16 * (g + 1), s], twrap[0:16, s]
                        )

                if ch_i % 4 != 3:
                    continue
                gi = ch_i // 4
                lo, hi = 12 * gi, 12 * gi + 12

                # ---- C-w: corner weights for tiles [lo, hi) ----
                # pyb/pxb carry the +16 shift AND the -0.4999999 floor bias,
                # so py here is (true py16 - 0.4999999) and floor is one magic
                # add, BIT-IDENTICAL to the wrapped-layout index path.
                dy = doff[:, lo:hi, 0:18:2]
                dx = doff[:, lo:hi, 1:18:2]
                py = tmp("py")[:, lo:hi]
                px = tmp("px")[:, lo:hi]
                nc.vector.tensor_tensor(py, pyb_sb[:, lo:hi], dy, Op.add)
                nc.vector.tensor_tensor(px, pxb_sb[:, lo:hi], dx, Op.add)
                y0 = tmp("y0")[:, lo:hi]
                x0 = tmp("x0")[:, lo:hi]
                nc.vector.tensor_scalar(y0, py, MAGIC, -MAGIC, Op.add, Op.add)
                nc.vector.tensor_scalar(x0, px, MAGIC, -MAGIC, Op.add, Op.add)
                ly = tmp("ly")[:, lo:hi]
                lx = tmp("lx")[:, lo:hi]
                nc.vector.scalar_tensor_tensor(
                    ly, py, 0.4999999, y0, Op.add, Op.subtract
                )
                nc.vector.scalar_tensor_tensor(
                    lx, px, 0.4999999, x0, Op.add, Op.subtract
                )

                ta_ = tmp("ta")[:, lo:hi]
                tb_ = tmp("tb")[:, lo:hi]
                tc_ = tmp("tc")[:, lo:hi]
                td_ = tmp("td")[:, lo:hi]
                # y weights: wy0 = (1-ly)*[0<=y0<=95], wy1 = ly*[0<=y0+1<=95]
                # (all bounds shifted +16)
                nc.vector.tensor_scalar(ta_, y0, 16.0, None, Op.is_ge)
                nc.vector.tensor_scalar(tb_, y0, 111.0, None, Op.is_le)
                vy0 = tmp("vy0")[:, lo:hi]
                nc.vector.tensor_tensor(vy0, ta_, tb_, Op.mult)
                nc.vector.tensor_scalar(ta_, y0, 15.0, None, Op.is_ge)
                nc.vector.tensor_scalar(tb_, y0, 110.0, None, Op.is_le)
                vy1 = tmp("vy1")[:, lo:hi]
                nc.vector.tensor_tensor(vy1, ta_, tb_, Op.mult)
                wy0 = tmp("wy0")[:, lo:hi]
                wy1 = tmp("wy1")[:, lo:hi]
                nc.vector.tensor_scalar(tc_, ly, -1.0, 1.0, Op.mult, Op.add)
                nc.vector.tensor_tensor(wy0, tc_, vy0, Op.mult)
                nc.vector.tensor_tensor(wy1, ly, vy1, Op.mult)

                # x pair weights on tokens (xb, xb+1), xb = clip(x0,0,95):
                # wA = (1-lx)*[0<=x0<=95] + lx*[x0==-1] ; wB = lx*[0<=x0<=94]
                nc.vector.tensor_scalar(ta_, x0, 16.0, None, Op.is_ge)
                nc.vector.tensor_scalar(tb_, x0, 111.0, None, Op.is_le)
                vx = tmp("vx")[:, lo:hi]
                nc.vector.tensor_tensor(vx, ta_, tb_, Op.mult)
                nc.vector.tensor_scalar(tb_, x0, 110.0, None, Op.is_le)
                vxb = tmp("vxb")[:, lo:hi]
                nc.vector.tensor_tensor(vxb, ta_, tb_, Op.mult)
                nc.vector.tensor_scalar(td_, x0, 15.0, None, Op.is_equal)
                wa = tmp("wa")[:, lo:hi]
                wb = tmp("wb")[:, lo:hi]
                nc.vector.tensor_scalar(tc_, lx, -1.0, 1.0, Op.mult, Op.add)
                nc.vector.tensor_tensor(tc_, tc_, vx, Op.mult)
                nc.vector.tensor_tensor(td_, lx, td_, Op.mult)
                nc.vector.tensor_tensor(wa, tc_, td_, Op.add)
                nc.vector.tensor_tensor(wb, lx, vxb, Op.mult)

                # final 4 corner weights matching the xt2 gather slot order
                # (y0,xb) (y1,xb) (y0,xb+1) (y1,xb+1)
                nc.vector.tensor_tensor(wt[:, lo:hi, :, 0], wy0, wa, Op.mult)
                nc.vector.tensor_tensor(wt[:, lo:hi, :, 1], wy1, wa, Op.mult)
                nc.vector.tensor_tensor(wt[:, lo:hi, :, 2], wy0, wb, Op.mult)
                nc.vector.tensor_tensor(wt[:, lo:hi, :, 3], wy1, wb, Op.mult)


        # ---- E: main loop ----
        with (
            tc.tile_pool(name="gpool", bufs=4) as gpool,
            tc.tile_pool(name="vpool", bufs=4) as vpool,
            tc.tile_pool(name="rpool", bufs=3) as rpool,
            tc.tile_pool(name="opool", bufs=3) as opool,
            tc.tile_pool(name="psT", bufs=4, space="PSUM") as pst,
            tc.tile_pool(name="psO", bufs=2, space="PSUM") as pso,
        ):
            # overlapped-window view of the row-pair table: [XT2R, 1024] stride 512
            xt2_ap = xt2
            xt2_win = dataclasses.replace(
                xt2_ap, ap=[[2 * C, XT2R], [1, 4 * C]], offset=0
            )
            for s in range(NSTAGE):
                po = [pso.tile([P, SPX], F32, tag=f"po{oh}", name=f"po{oh}") for oh in range(2)]
                for tap in range(K2):
                    g = gpool.tile([P, TPS, 4 * C], BF, tag="g", name="g")
                    idxs = twrap[:, s, tap]
                    nc.gpsimd.dma_gather(
                        g[:],
                        xt2_win,
                        idxs,
                        SPX,
                        SPX,
                        elem_size=4 * C,
                        elem_step=2 * C,
                        queue_num=tap % 2,
                    )
                    rst = [rpool.tile([P, SPX], BF, tag=f"r{c}", name=f"r{c}") for c in range(2)]
                    for t in range(TPS):
                        v = vpool.tile([P, C], BF, tag="v", name="v")
                        wcol = wt[:, s * TPS + t, tap, :]
                        nc.scalar.activation(
                            v[:], g[:, t, 0:C], Act.Identity,
                            scale=wcol[:, 0:1],
                        )
                        nc.vector.scalar_tensor_tensor(
                            v[:], g[:, t, C : 2 * C], wcol[:, 1:2], v[:],
                            Op.mult, Op.add,
                        )
                        nc.vector.scalar_tensor_tensor(
                            v[:], g[:, t, 2 * C : 3 * C], wcol[:, 2:3], v[:],
                            Op.mult, Op.add,
                        )
                        nc.vector.scalar_tensor_tensor(
                            v[:], g[:, t, 3 * C : 4 * C], wcol[:, 3:4], v[:],
                            Op.mult, Op.add,
                        )
                        for chalf in range(2):
                            ptr = pst.tile([P, P], BF, tag="ptr", name="ptr")
                            nc.tensor.transpose(
                                ptr[:],
                                v[:, chalf * P : (chalf + 1) * P],
                                ident_bf[:],
                            )
                            nc.scalar.copy(
                                rst[chalf][:, t * P : (t + 1) * P], ptr[:]
                            )
                    for chalf in range(2):
                        for oh in range(2):
                            nc.tensor.matmul(
                                po[oh][:],
                                wdcl_sb[:, tap, chalf, oh],
                                rst[chalf][:],
                                start=(tap == 0 and chalf == 0),
                                stop=(tap == K2 - 1 and chalf == 1),
                            )
                for oh in range(2):
                    ob = opool.tile([P, SPX], F32, tag="ob", name="ob")
                    nc.scalar.activation(
                        ob[:], po[oh][:], Act.Identity, bias=bdc_sb[:, oh : oh + 1]
                    )
                    nc.sync.dma_start(
                        out[oh, :, s * SPX : (s + 1) * SPX], ob[:]
                    )


def _build():
    if "nc" in _BUILT:
        return _BUILT["nc"]
    nc = bacc.Bacc(
        "TRN2",
        target_bir_lowering=False,
        debug=False,
        enable_asserts=False,
        num_devices=NCORES,
        num_swdge_queues=2,
    )
    xt2 = nc.dram_tensor("xt2", [XT2R + 1, 2 * C], BF, kind="ExternalInput").ap()
    xc = nc.dram_tensor("xc", [P, 2, PADH * PADW], BF, kind="ExternalInput").ap()
    wofl = nc.dram_tensor("wofl", [P, 2, K2, 18], BF, kind="ExternalInput").ap()
    boff = nc.dram_tensor("boff", [18, 1], F32, kind="ExternalInput").ap()
    wdcl = nc.dram_tensor("wdcl", [P, K2, 2, 2, P], BF, kind="ExternalInput").ap()
    bdc = nc.dram_tensor("bdc", [P, 2], F32, kind="ExternalInput").ap()
    pyb = nc.dram_tensor("pyb", [P, NTILE, K2], F32, kind="ExternalInput").ap()
    pxb = nc.dram_tensor("pxb", [P, NTILE, K2], F32, kind="ExternalInput").ap()
    pybw = nc.dram_tensor("pybw", [16, NTILE, 8, K2], F32, kind="ExternalInput").ap()
    pxbw = nc.dram_tensor("pxbw", [16, NTILE, 8, K2], F32, kind="ExternalInput").ap()
    out = nc.dram_tensor("out", [2, P, NPIX], F32, kind="ExternalOutput").ap()
    with tile.TileContext(nc) as tc:
        _emit(tc, nc, (xt2, xc, wofl, boff, wdcl, bdc, pyb, pxb, pybw, pxbw, out))
    nc.compile()
    _BUILT["nc"] = nc
    return nc


def _make_xt2(xs):
    """xs: [C,H,W] f32 -> row-pair token table [XT2R, 2C] f32."""
    xp = np.zeros((XPAD, C), np.float32)
    xp[96 : 96 + H * W] = xs.transpose(1, 2, 0).reshape(H * W, C)
    return np.concatenate([xp[: XT2R + 1], xp[96 : 96 + XT2R + 1]], axis=1)


def _prep_core(k, x, w_off, b_off, w_dc, b_dc, xt2_cache):
    b, half = k // 2, k % 2
    y0 = half * ROWS
    xs = x[b]  # [C,H,W] f32
    if b not in xt2_cache:
        xt2_cache[b] = _make_xt2(xs)
    xt2 = xt2_cache[b]
    xc = np.zeros((C, PADH, PADW), np.float32)
    r0, r1 = max(0, y0 - 1), min(H, y0 + ROWS + 1)
    xc[:, (r0 - (y0 - 1)) : (r1 - (y0 - 1)), 1 : 1 + W] = xs[:, r0:r1, :]
    xc = xc.reshape(2, P, PADH * PADW).transpose(1, 0, 2)

    wofl = (
        w_off.reshape(2 * K2, 2, P, K2)   # [oc, chalf, c, tap]
        .transpose(2, 1, 3, 0)            # [c, chalf, tap, oc]
        .copy()
    )
    wdcl = (
        w_dc.reshape(2, P, 2, P, K2)      # [oh, o, chalf, c, tap]
        .transpose(3, 4, 2, 0, 1)         # [c, tap, chalf, oh, o]
        .copy()
    )
    bdc = b_dc.reshape(2, P).transpose(1, 0).copy()

    ti = (np.arange(K2) // K)
    tj = (np.arange(K2) % K)

    pp = np.arange(NPIX)
    yg = y0 + pp // W
    xg = pp % W
    pyb = (yg[:, None] - 1 + ti[None, :]
           + 16.0 - 0.4999999).astype(np.float32).reshape(NTILE, P, K2)
    pxb = (xg[:, None] - 1 + tj[None, :]
           + 16.0 - 0.4999999).astype(np.float32).reshape(NTILE, P, K2)

    # wrapped-layout base tables [16r, (t, m, tap)]: pixel = t*128 + 16m + r,
    # +16 shift and -0.4999999 floor-bias baked in.
    t_i = np.arange(NTILE)
    m_i = np.arange(8)
    r_i = np.arange(16)
    pw = (t_i[None, :, None] * P + 16 * m_i[None, None, :]
          + r_i[:, None, None])                      # [16, 36, 8]
    ygw = y0 + pw // W
    xgw = pw % W
    pybw = (ygw[..., None] - 1 + ti[None, None, None, :]
            + 16.0 - 0.4999999).astype(np.float32)   # [16, 36, 8, 9]
    pxbw = (xgw[..., None] - 1 + tj[None, None, None, :]
            + 16.0 - 0.4999999).astype(np.float32)

    import ml_dtypes

    bf16 = ml_dtypes.bfloat16
    return {
        "xt2": xt2.astype(bf16),
        "xc": xc.astype(bf16),
        "wofl": wofl.astype(bf16),
        "boff": b_off.reshape(18, 1).astype(np.float32),
        "wdcl": wdcl.astype(bf16),
        "bdc": bdc.astype(np.float32),
        "pyb": pyb.transpose(1, 0, 2).copy(),
        "pxb": pxb.transpose(1, 0, 2).copy(),
        "pybw": pybw,
        "pxbw": pxbw,
    }


def kernel(x, w_off, b_off, w_dc, b_dc, _trace=False):
    nc = _build()
    x = np.asarray(x, np.float32)
    w_off = np.asarray(w_off, np.float32)
    b_off = np.asarray(b_off, np.float32)
    w_dc = np.asarray(w_dc, np.float32)
    b_dc = np.asarray(b_dc, np.float32)
    xt2_cache = {}
    in_maps = [
        _prep_core(k, x, w_off, b_off, w_dc, b_dc, xt2_cache)
        for k in range(NCORES)
    ]
    res = bass_utils.run_bass_kernel_spmd(
        nc, in_maps, core_ids=list(range(NCORES)), trace=_trace
    )
    out = np.empty((B, O, H, W), np.float32)
    for k in range(NCORES):
        b, half = k // 2, k % 2
        o = res.results[k]["out"]  # [2,128,4608]
        out[b, :, half * ROWS : (half + 1) * ROWS, :] = o.reshape(
            O, ROWS, W
        )
    if _trace:
        return out, res
    return out


# revision 19
# speedup vs baseline: 1.0585x; 1.0585x over previous
"""Deformable conv (3x3, with offset-predicting conv) for Trainium2, 8 cores.

Sharding: pure data parallel. Core k handles sample b = k//2, output row block
(k%2)*48 .. +48 (48 rows x 96 cols = 4608 pixels). Full sample's x is available
to every core as a DRAM row-pair token table, so gathers are purely local.

Per-core pipeline (all on one NeuronCore, scheduled by Tile):
  A. offset conv (3x3, C=256 -> 18) as 18 PE matmuls per 4-row chunk
  B. PE-transpose offsets into pixel-partition layout [128px, tile, 18]
  C-w. DVE weight math in pixel layout: 4 bilinear corner weights with
     zero-pad semantics exactly like the reference -> wt[:, t, tap, 4]
  D. fold offsets (f32, 72B beats) into the SWDGE "wrapped 16-partition"
     layout doffW0[r, (t, m, 18)] via 8 DMAs; compute gather base indices
     directly in wrapped layout on the DVE; int16-copy into the
     (s, tap, t', m) stream order; replicate x8.
  E. per (stage of 512 px, tap): ONE dma_gather of 2KB 4-corner descriptors
     from the row-pair table xt2 -> [128px, tile, 4*C]; DVE 4-term FMA
     combine with per-partition (per-pixel) scalar weights; PE transpose to
     channel layout; PE matmul accumulating over (c,tap) into PSUM [o,px];
     bias + store.

The 4-corner trick: xt2[i] = concat(xflat[i], xflat[i+96]) over a zero-padded
token space (1 pad row above, 3 below), so one contiguous 1024-element read at
row base2 = (y0+1)*96 + xb yields all four bilinear corners
(y0,xb),(y1,xb),(y0,xb+1),(y1,xb+1).
"""

import dataclasses

import numpy as np

import concourse.bacc as bacc
import concourse.bass as bass
import concourse.mybir as mybir
import concourse.tile as tile
from concourse import bass_utils, masks
from concourse.mybir import ActivationFunctionType as Act
from concourse.mybir import AluOpType as Op

P = 128
B, C, H, W, O = 4, 256, 96, 96, 256
K = 3
K2 = 9
NCORES = 8
ROWS = 48                      # output rows per core
NPIX = ROWS * W                # 4608
NTILE = NPIX // P              # 36 pixel tiles of 128
NSTAGE = 9                     # stages of 512 px
TPS = 4                        # pixel tiles per stage
SPX = TPS * P                  # 512
PADH, PADW = ROWS + 2, W + 2   # 50, 98
XT2R = 9412                    # row-pair table rows (max base2 9407, +1, pad)
XPAD = 9600                    # padded flat token space (96 above, 288 below)
CONV_ROWS_PER_CHUNK = 4        # offset-conv N chunk = 4 rows = 384 cols
NCHUNK = ROWS // CONV_ROWS_PER_CHUNK  # 12
NW = NTILE * 8 * K2            # 2592 wrapped-layout elems per idx partition
BF = mybir.dt.bfloat16
F32 = mybir.dt.float32
I16 = mybir.dt.int16

_BUILT = {}


def _emit(tc, nc, io):
    xt2, xc, wofl, boff, wdcl, bdc, pyb, pxb, pybw, pxbw, out = io

    with (
        tc.tile_pool(name="const", bufs=1) as cpool,
        tc.tile_pool(name="sbig", bufs=1) as spool,
    ):
        ident_bf = cpool.tile([P, P], BF, tag="idbf", name="idbf")
        ident_f = cpool.tile([P, P], F32, tag="idf", name="idf")
        masks.make_identity(nc, ident_bf[:])
        masks.make_identity(nc, ident_f[:])

        # ---- persistent SBUF buffers ----
        xc_sb = spool.tile([P, 2, PADH * PADW], BF, tag="xc", name="xc")      # 19.6KB/part
        wofl_sb = spool.tile([P, 2, K2, 18], BF, tag="wofl", name="wofl")
        wdcl_sb = spool.tile([P, K2, 2, 2, P], BF, tag="wdcl", name="wdcl")     # 9.2KB/part
        boff_sb = spool.tile([18, 1], F32, tag="boff", name="boff")
        bdc_sb = spool.tile([P, 2], F32, tag="bdc", name="bdc")
        pyb_sb = spool.tile([P, NTILE, K2], F32, tag="pyb", name="pyb")
        pxb_sb = spool.tile([P, NTILE, K2], F32, tag="pxb", name="pxb")
        pybw_sb = spool.tile([16, NTILE, 8, K2], F32, tag="pybw", name="pybw")
        pxbw_sb = spool.tile([16, NTILE, 8, K2], F32, tag="pxbw", name="pxbw")
        off_sb = spool.tile([18, NPIX], F32, tag="off", name="off")            # 18 parts
        doff = spool.tile([P, NTILE, 18], F32, tag="doff", name="doff")
        doffw = spool.tile([16, NTILE, 8, 18], F32, tag="doffw", name="doffw")  # 20.7KB
        wt = spool.tile([P, NTILE, K2, 4], F32, tag="wt", name="wt")          # corner wgts
        twrap = spool.tile([P, NSTAGE, K2, TPS, 8], I16, tag="twrap", name="twrap")  # 5.2KB

        nc.sync.dma_start(xc_sb[:], xc)
        nc.sync.dma_start(wofl_sb[:], wofl)
        nc.sync.dma_start(wdcl_sb[:], wdcl)
        nc.sync.dma_start(boff_sb[:], boff)
        nc.sync.dma_start(bdc_sb[:], bdc)
        nc.sync.dma_start(pyb_sb[:], pyb)
        nc.sync.dma_start(pxb_sb[:], pxb)
        nc.sync.dma_start(pybw_sb[:], pybw)
        nc.sync.dma_start(pxbw_sb[:], pxbw)

        # ---- A/B/C/D pipelined per group of 4 conv chunks (12 tiles) ----
        # A: offset conv chunk (4 rows); B: PE-transpose its 3 tiles; after
        # each group of 4 chunks: C-w corner weights for those 12 tiles, fold
        # offsets into wrapped layout, then per-stage index math + replicate,
        # so stage-0 gathers can start while later conv chunks still run.
        MAGIC = 8388608.0  # 2^23
        dsrc = doff[:]
        pitch_s = dsrc.ap[0][0]
        ddst = doffw[:]
        pitch_d = ddst.ap[0][0]
        with (
            tc.tile_pool(name="psA", bufs=2, space="PSUM") as psa,
            tc.tile_pool(name="psB", bufs=4, space="PSUM") as psb,
            tc.tile_pool(name="scr", bufs=1) as scr,
            tc.tile_pool(name="scw", bufs=1) as scw,
        ):
            sh = [P, NTILE, K2]

            def tmp(tag):
                return scr.tile(sh, F32, tag=tag, name=tag)

            pyw = scw.tile([16, NTILE, 8, K2], F32, tag="pyw", name="pyw")
            pxw = scw.tile([16, NTILE, 8, K2], F32, tag="pxw", name="pxw")

            for ch_i in range(NCHUNK):
                ncols = CONV_ROWS_PER_CHUNK * W  # 384
                ps = psa.tile([18, ncols], F32, tag="psoff", name="psoff")
                n_mm = 2 * K2
                mm = 0
                xcf = xc_sb[:]
                for chalf in range(2):
                    for tap in range(K2):
                        ti, tj = tap // K, tap % K
                        rhs = dataclasses.replace(
                            xcf,
                            ap=[
                                [xcf.ap[0][0], P],
                                [PADW, CONV_ROWS_PER_CHUNK],
                                [1, W],
                            ],
                            offset=xcf.offset
                            + chalf * (PADH * PADW)
                            + ((ch_i * CONV_ROWS_PER_CHUNK + ti) * PADW + tj),
                        )
                        nc.tensor.matmul(
                            ps[:],
                            wofl_sb[:, chalf, tap],
                            rhs,
                            start=(mm == 0),
                            stop=(mm == n_mm - 1),
                        )
                        mm += 1
                nc.scalar.activation(
                    off_sb[:, ch_i * ncols : (ch_i + 1) * ncols],
                    ps[:],
                    Act.Identity,
                    bias=boff_sb[:],
                )
                # B: transpose this chunk's 3 tiles to pixel layout
                for t in range(3 * ch_i, 3 * ch_i + 3):
                    pt = psb.tile([P, 18], F32, tag="pofft", name="pofft")
                    nc.tensor.transpose(
                        pt[:], off_sb[:, t * P : (t + 1) * P], ident_f[:18, :18]
                    )
                    nc.scalar.copy(doff[:, t, :], pt[:])

                if ch_i % 4 != 3:
                    continue
                gi = ch_i // 4
                lo, hi = 12 * gi, 12 * gi + 12

                # ---- C-w: corner weights for tiles [lo, hi) ----
                # pyb/pxb carry the +16 shift AND the -0.4999999 floor bias,
                # so py here is (true py16 - 0.4999999) and floor is one magic
                # add, BIT-IDENTICAL to the wrapped-layout index path.
                dy = doff[:, lo:hi, 0:18:2]
                dx = doff[:, lo:hi, 1:18:2]
                py = tmp("py")[:, lo:hi]
                px = tmp("px")[:, lo:hi]
                nc.vector.tensor_tensor(py, pyb_sb[:, lo:hi], dy, Op.add)
                nc.vector.tensor_tensor(px, pxb_sb[:, lo:hi], dx, Op.add)
                y0 = tmp("y0")[:, lo:hi]
                x0 = tmp("x0")[:, lo:hi]
                nc.vector.tensor_scalar(y0, py, MAGIC, -MAGIC, Op.add, Op.add)
                nc.vector.tensor_scalar(x0, px, MAGIC, -MAGIC, Op.add, Op.add)
                ly = tmp("ly")[:, lo:hi]
                lx = tmp("lx")[:, lo:hi]
                nc.vector.scalar_tensor_tensor(
                    ly, py, 0.4999999, y0, Op.add, Op.subtract
                )
                nc.vector.scalar_tensor_tensor(
                    lx, px, 0.4999999, x0, Op.add, Op.subtract
                )

                ta_ = tmp("ta")[:, lo:hi]
                tb_ = tmp("tb")[:, lo:hi]
                tc_ = tmp("tc")[:, lo:hi]
                td_ = tmp("td")[:, lo:hi]
                # y weights: wy0 = (1-ly)*[0<=y0<=95], wy1 = ly*[0<=y0+1<=95]
                # (all bounds shifted +16)
                nc.vector.tensor_scalar(ta_, y0, 16.0, None, Op.is_ge)
                nc.vector.tensor_scalar(tb_, y0, 111.0, None, Op.is_le)
                vy0 = tmp("vy0")[:, lo:hi]
                nc.vector.tensor_tensor(vy0, ta_, tb_, Op.mult)
                nc.vector.tensor_scalar(ta_, y0, 15.0, None, Op.is_ge)
                nc.vector.tensor_scalar(tb_, y0, 110.0, None, Op.is_le)
                vy1 = tmp("vy1")[:, lo:hi]
                nc.vector.tensor_tensor(vy1, ta_, tb_, Op.mult)
                wy0 = tmp("wy0")[:, lo:hi]
                wy1 = tmp("wy1")[:, lo:hi]
                nc.vector.tensor_scalar(tc_, ly, -1.0, 1.0, Op.mult, Op.add)
                nc.vector.tensor_tensor(wy0, tc_, vy0, Op.mult)
                nc.vector.tensor_tensor(wy1, ly, vy1, Op.mult)

                # x pair weights on tokens (xb, xb+1), xb = clip(x0,0,95):
                # wA = (1-lx)*[0<=x0<=95] + lx*[x0==-1] ; wB = lx*[0<=x0<=94]
                nc.vector.tensor_scalar(ta_, x0, 16.0, None, Op.is_ge)
                nc.vector.tensor_scalar(tb_, x0, 111.0, None, Op.is_le)
                vx = tmp("vx")[:, lo:hi]
                nc.vector.tensor_tensor(vx, ta_, tb_, Op.mult)
                nc.vector.tensor_scalar(tb_, x0, 110.0, None, Op.is_le)
                vxb = tmp("vxb")[:, lo:hi]
                nc.vector.tensor_tensor(vxb, ta_, tb_, Op.mult)
                nc.vector.tensor_scalar(td_, x0, 15.0, None, Op.is_equal)
                wa = tmp("wa")[:, lo:hi]
                wb = tmp("wb")[:, lo:hi]
                nc.vector.tensor_scalar(tc_, lx, -1.0, 1.0, Op.mult, Op.add)
                nc.vector.tensor_tensor(tc_, tc_, vx, Op.mult)
                nc.vector.tensor_tensor(td_, lx, td_, Op.mult)
                nc.vector.tensor_tensor(wa, tc_, td_, Op.add)
                nc.vector.tensor_tensor(wb, lx, vxb, Op.mult)

                # final 4 corner weights matching the xt2 gather slot order
                # (y0,xb) (y1,xb) (y0,xb+1) (y1,xb+1)
                nc.vector.tensor_tensor(wt[:, lo:hi, :, 0], wy0, wa, Op.mult)
                nc.vector.tensor_tensor(wt[:, lo:hi, :, 1], wy1, wa, Op.mult)
                nc.vector.tensor_tensor(wt[:, lo:hi, :, 2], wy0, wb, Op.mult)
                nc.vector.tensor_tensor(wt[:, lo:hi, :, 3], wy1, wb, Op.mult)

                # ---- fold offsets into wrapped layout for tiles [lo, hi) ----
                # doffw[r, t, m, ch] = doff[16m + r, t, ch] (72B beats)
                for m in range(8):
                    src = dataclasses.replace(
                        dsrc,
                        ap=[[pitch_s, 16], [1, 12 * 18]],
                        offset=dsrc.offset + 16 * m * pitch_s + lo * 18,
                    )
                    dst = dataclasses.replace(
                        ddst,
                        ap=[[pitch_d, 16], [8 * 18, 12], [1, 18]],
                        offset=ddst.offset + lo * 8 * 18 + m * 18,
                    )
                    nc.sync.dma_start(dst, src)

                # ---- per-stage index math + int16 stream copy + replicate ----
                # base2 = (clip(y0,-1,96)+1)*96 + clip(x0,0,95), with the +16
                # shift and floor bias baked into pybw/pxbw.
                for s in range(3 * gi, 3 * gi + 3):
                    sl = slice(TPS * s, TPS * (s + 1))
                    pys = pyw[:, sl]
                    pxs = pxw[:, sl]
                    nc.vector.tensor_tensor(
                        pys, pybw_sb[:, sl], doffw[:, sl, :, 0:18:2], Op.add
                    )
                    nc.vector.tensor_tensor(
                        pxs, pxbw_sb[:, sl], doffw[:, sl, :, 1:18:2], Op.add
                    )
                    nc.vector.tensor_scalar(pys, pys, MAGIC, -MAGIC, Op.add, Op.add)
                    nc.vector.tensor_scalar(pxs, pxs, MAGIC, -MAGIC, Op.add, Op.add)
                    nc.vector.tensor_scalar(pys, pys, 15.0, 112.0, Op.max, Op.min)
                    nc.vector.tensor_scalar(pxs, pxs, 16.0, 111.0, Op.max, Op.min)
                    nc.vector.tensor_scalar(pys, pys, 96.0, -1456.0, Op.mult, Op.add)
                    nc.vector.tensor_tensor(pys, pys, pxs, Op.add)
                    nc.vector.tensor_copy(
                        twrap[0:16, s].transpose((0, 2, 3, 1)), pys
                    )
                    for g in range(1, 8):
                        nc.sync.dma_start(
                            twrap[16 * g : 16 * (g + 1), s], twrap[0:16, s]
                        )

        # ---- E: main loop ----
        with (
            tc.tile_pool(name="gpool", bufs=4) as gpool,
            tc.tile_pool(name="vpool", bufs=4) as vpool,
            tc.tile_pool(name="rpool", bufs=3) as rpool,
            tc.tile_pool(name="opool", bufs=3) as opool,
            tc.tile_pool(name="psT", bufs=4, space="PSUM") as pst,
            tc.tile_pool(name="psO", bufs=2, space="PSUM") as pso,
        ):
            # overlapped-window view of the row-pair table: [XT2R, 1024] stride 512
            xt2_ap = xt2
            xt2_win = dataclasses.replace(
                xt2_ap, ap=[[2 * C, XT2R], [1, 4 * C]], offset=0
            )
            for s in range(NSTAGE):
                po = [pso.tile([P, SPX], F32, tag=f"po{oh}", name=f"po{oh}") for oh in range(2)]
                for tap in range(K2):
                    g = gpool.tile([P, TPS, 4 * C], BF, tag="g", name="g")
                    idxs = twrap[:, s, tap]
                    nc.gpsimd.dma_gather(
                        g[:],
                        xt2_win,
                        idxs,
                        SPX,
                        SPX,
                        elem_size=4 * C,
                        elem_step=2 * C,
                        queue_num=tap % 2,
                    )
                    rst = [rpool.tile([P, SPX], BF, tag=f"r{c}", name=f"r{c}") for c in range(2)]
                    for t in range(TPS):
                        v = vpool.tile([P, C], BF, tag="v", name="v")
                        wcol = wt[:, s * TPS + t, tap, :]
                        nc.scalar.activation(
                            v[:], g[:, t, 0:C], Act.Identity,
                            scale=wcol[:, 0:1],
                        )
                        nc.vector.scalar_tensor_tensor(
                            v[:], g[:, t, C : 2 * C], wcol[:, 1:2], v[:],
                            Op.mult, Op.add,
                        )
                        nc.vector.scalar_tensor_tensor(
                            v[:], g[:, t, 2 * C : 3 * C], wcol[:, 2:3], v[:],
                            Op.mult, Op.add,
                        )
                        nc.vector.scalar_tensor_tensor(
                            v[:], g[:, t, 3 * C : 4 * C], wcol[:, 3:4], v[:],
                            Op.mult, Op.add,
                        )
                        for chalf in range(2):
                            ptr = pst.tile([P, P], BF, tag="ptr", name="ptr")
                            nc.tensor.transpose(
                                ptr[:],
                                v[:, chalf * P : (chalf + 1) * P],
                                ident_bf[:],
                            )
                            nc.scalar.copy(
                                rst[chalf][:, t * P : (t + 1) * P], ptr[:]
                            )
                    for chalf in range(2):
                        for oh in range(2):
                            nc.tensor.matmul(
                                po[oh][:],
                                wdcl_sb[:, tap, chalf, oh],
                                rst[chalf][:],
                                start=(tap == 0 and chalf == 0),
                                stop=(tap == K2 - 1 and chalf == 1),
                            )
                for oh in range(2):
                    ob = opool.tile([P, SPX], F32, tag="ob", name="ob")
                    nc.scalar.activation(
                        ob[:], po[oh][:], Act.Identity, bias=bdc_sb[:, oh : oh + 1]
                    )
                    nc.sync.dma_start(
                        out[oh, :, s * SPX : (s + 1) * SPX], ob[:]
                    )


def _build():
    if "nc" in _BUILT:
        return _BUILT["nc"]
    nc = bacc.Bacc(
        "TRN2",
        target_bir_lowering=False,
        debug=False,
        enable_asserts=False,
        num_devices=NCORES,
        num_swdge_queues=2,
    )
    xt2 = nc.dram_tensor("xt2", [XT2R + 1, 2 * C], BF, kind="ExternalInput").ap()
    xc = nc.dram_tensor("xc", [P, 2, PADH * PADW], BF, kind="ExternalInput").ap()
    wofl = nc.dram_tensor("wofl", [P, 2, K2, 18], BF, kind="ExternalInput").ap()
    boff = nc.dram_tensor("boff", [18, 1], F32, kind="ExternalInput").ap()
    wdcl = nc.dram_tensor("wdcl", [P, K2, 2, 2, P], BF, kind="ExternalInput").ap()
    bdc = nc.dram_tensor("bdc", [P, 2], F32, kind="ExternalInput").ap()
    pyb = nc.dram_tensor("pyb", [P, NTILE, K2], F32, kind="ExternalInput").ap()
    pxb = nc.dram_tensor("pxb", [P, NTILE, K2], F32, kind="ExternalInput").ap()
    pybw = nc.dram_tensor("pybw", [16, NTILE, 8, K2], F32, kind="ExternalInput").ap()
    pxbw = nc.dram_tensor("pxbw", [16, NTILE, 8, K2], F32, kind="ExternalInput").ap()
    out = nc.dram_tensor("out", [2, P, NPIX], F32, kind="ExternalOutput").ap()
    with tile.TileContext(nc) as tc:
        _emit(tc, nc, (xt2, xc, wofl, boff, wdcl, bdc, pyb, pxb, pybw, pxbw, out))
    nc.compile()
    _BUILT["nc"] = nc
    return nc


def _make_xt2(xs):
    """xs: [C,H,W] f32 -> row-pair token table [XT2R, 2C] f32."""
    xp = np.zeros((XPAD, C), np.float32)
    xp[96 : 96 + H * W] = xs.transpose(1, 2, 0).reshape(H * W, C)
    return np.concatenate([xp[: XT2R + 1], xp[96 : 96 + XT2R + 1]], axis=1)


def _prep_core(k, x, w_off, b_off, w_dc, b_dc, xt2_cache):
    b, half = k // 2, k % 2
    y0 = half * ROWS
    xs = x[b]  # [C,H,W] f32
    if b not in xt2_cache:
        xt2_cache[b] = _make_xt2(xs)
    xt2 = xt2_cache[b]
    xc = np.zeros((C, PADH, PADW), np.float32)
    r0, r1 = max(0, y0 - 1), min(H, y0 + ROWS + 1)
    xc[:, (r0 - (y0 - 1)) : (r1 - (y0 - 1)), 1 : 1 + W] = xs[:, r0:r1, :]
    xc = xc.reshape(2, P, PADH * PADW).transpose(1, 0, 2)

    wofl = (
        w_off.reshape(2 * K2, 2, P, K2)   # [oc, chalf, c, tap]
        .transpose(2, 1, 3, 0)            # [c, chalf, tap, oc]
        .copy()
    )
    wdcl = (
        w_dc.reshape(2, P, 2, P, K2)      # [oh, o, chalf, c, tap]
        .transpose(3, 4, 2, 0, 1)         # [c, tap, chalf, oh, o]
        .copy()
    )
    bdc = b_dc.reshape(2, P).transpose(1, 0).copy()

    ti = (np.arange(K2) // K)
    tj = (np.arange(K2) % K)

    pp = np.arange(NPIX)
    yg = y0 + pp // W
    xg = pp % W
    pyb = (yg[:, None] - 1 + ti[None, :]
           + 16.0 - 0.4999999).astype(np.float32).reshape(NTILE, P, K2)
    pxb = (xg[:, None] - 1 + tj[None, :]
           + 16.0 - 0.4999999).astype(np.float32).reshape(NTILE, P, K2)

    # wrapped-layout base tables [16r, (t, m, tap)]: pixel = t*128 + 16m + r,
    # +16 shift and -0.4999999 floor-bias baked in.
    t_i = np.arange(NTILE)
    m_i = np.arange(8)
    r_i = np.arange(16)
    pw = (t_i[None, :, None] * P + 16 * m_i[None, None, :]
          + r_i[:, None, None])                      # [16, 36, 8]
    ygw = y0 + pw // W
    xgw = pw % W
    pybw = (ygw[..., None] - 1 + ti[None, None, None, :]
            + 16.0 - 0.4999999).astype(np.float32)   # [16, 36, 8, 9]
    pxbw = (xgw[..., None] - 1 + tj[None, None, None, :]
            + 16.0 - 0.4999999).astype(np.float32)

    import ml_dtypes

    bf16 = ml_dtypes.bfloat16
    return {
        "xt2": xt2.astype(bf16),
        "xc": xc.astype(bf16),
        "wofl": wofl.astype(bf16),
        "boff": b_off.reshape(18, 1).astype(np.float32),
        "wdcl": wdcl.astype(bf16),
        "bdc": bdc.astype(np.float32),
        "pyb": pyb.transpose(1, 0, 2).copy(),
        "pxb": pxb.transpose(1, 0, 2).copy(),
        "pybw": pybw,
        "pxbw": pxbw,
    }


def kernel(x, w_off, b_off, w_dc, b_dc, _trace=False):
    nc = _build()
    x = np.asarray(x, np.float32)
    w_off = np.asarray(w_off, np.float32)
    b_off = np.asarray(b_off, np.float32)
    w_dc = np.asarray(w_dc, np.float32)
    b_dc = np.asarray(b_dc, np.float32)
    xt2_cache = {}
    in_maps = [
        _prep_core(k, x, w_off, b_off, w_dc, b_dc, xt2_cache)
        for k in range(NCORES)
    ]
    res = bass_utils.run_bass_kernel_spmd(
        nc, in_maps, core_ids=list(range(NCORES)), trace=_trace
    )
    out = np.empty((B, O, H, W), np.float32)
    for k in range(NCORES):
        b, half = k // 2, k % 2
        o = res.results[k]["out"]  # [2,128,4608]
        out[b, :, half * ROWS : (half + 1) * ROWS, :] = o.reshape(
            O, ROWS, W
        )
    if _trace:
        return out, res
    return out
